# revision 37
# baseline (speedup 1.0000x reference)
"""Trainium2 Bass kernel for blocked-DCT high-frequency extractor.

Computes, for x (64, 3, 512, 512) f32:
  gray = 0.299*R + 0.587*G + 0.114*B                     (B,1,H,W)
  per 8x8 block:  Y = mask * (D @ block @ D.T)           (2D DCT + high-pass)
  output (64, 1, 512, 512) f32

Strategy (pure data parallel over batch, 8 batches/core on 8 cores; the
kernel is HBM-bound: 24 MiB in + 8 MiB out per core, ~298 GB/s/core
achievable with all 8 cores running => ~113 us floor).

Per core, per (batch, 128-row chunk) of the image:
  1. One fused 768 KB DMA on the SP HWDGE queue brings all 3 channel
     chunks into a (128h, 3*512w) tile (2 KB contiguous runs).
  2. Grayscale spread over three engines so none saturates:
     g0 = x0*(w0/w2) + x2 on DVE (scalar_tensor_tensor),
     gs = x1*(w1/w2) on ACT, g1 = g0 + gs on GpSimd.
  3. H-direction DCT: one matmul with sqrt(w2) * (I_16 kron D^T).
  4. DVE stream-transpose (independent 32x32 blocks) read straight from
     PSUM. Because 8 | 32, this puts w%32 (which contains the
     intra-block w index b) on partitions.
  5. W-direction DCT: one matmul with the same stationary weight
     (the two sqrt(w2) factors give the grayscale w2 scale in total).
  6. High-pass mask on ACT as two strided PSUM->SBUF copies: columns
     with u<4 are scaled by a per-partition 0/1 vector (zero iff v<4),
     u>=4 columns are a plain copy. This keeps the mask off the DVE,
     whose two structural transposes are the tightest compute budget.
  7. DVE stream-transpose back -> exact (hfreq, wfreq) output layout.
  8. 256 KB contiguous output DMA on the ACT HWDGE queue (separate
     queue from the input stream).

The 32x32 block transpose is an involution whose block-nesting (8 | 32)
makes both DCT matmuls use the same I_16 kron D^T stationary weight and
lands the final result in natural row-major layout with zero TensorE
transposes.
"""

import os

import numpy as np

import concourse.bacc as bacc
import concourse.mybir as mybir
import concourse.tile as tile
from concourse.bass_utils import run_bass_kernel_spmd

N_CORES = 8
B, C, H, W = 64, 3, 512, 512
BLOC = B // N_CORES  # batches per core
P = 128              # SBUF partitions / chunk height
NCH = H // P         # 128-row chunks per image
F32 = mybir.dt.float32
GRAY_W = (0.299, 0.587, 0.114)

_NC = None          # cached compiled Bass module
LAST_RUN = None     # BassKernelResults of the most recent run (for test.py)


def _build_bass():
    nc = bacc.Bacc(
        "TRN2",
        target_bir_lowering=False,
        debug=False,
        num_devices=N_CORES,
    )
    x = nc.declare_dram_parameter("x", [BLOC, C, H, W], F32, isOutput=False)
    wts = nc.declare_dram_parameter("wts", [1, P, P], F32, isOutput=False)
    mvec = nc.declare_dram_parameter("mvec", [P, 1], F32, isOutput=False)
    out = nc.declare_dram_parameter("out", [BLOC, 1, H, W], F32, isOutput=True)

    # gray = GW[2] * (x0*(w0/w2) + x2  +  x1*(w1/w2)); the GW[2] scale is
    # folded into the mask tile on the host side. The two scaled terms are
    # computed on different engines in parallel, then summed on GpSimd.
    ga = GRAY_W[0] / GRAY_W[2]
    gb = GRAY_W[1] / GRAY_W[2]
    mult = mybir.AluOpType.mult
    add = mybir.AluOpType.add

    with tile.TileContext(nc) as tc:
        with (
            tc.tile_pool(name="consts", bufs=1) as consts,
            tc.tile_pool(name="xin", bufs=8) as xin,
            tc.tile_pool(name="work", bufs=6) as work,
            tc.tile_pool(name="psum", bufs=4, space="PSUM") as psum_pool,
        ):
            wd = consts.tile([P, P], F32, tag="wd")
            nc.sync.dma_start(wd[:], wts[0])
            mv = consts.tile([P, 1], F32, tag="mvec")
            nc.sync.dma_start(mv[:], mvec[:])

            # out-DMA for chunk i is emitted at the top of iteration i+1 so
            # ACT's in-order stream never parks on the wait for DVE's final
            # transpose ahead of the next chunk's compute ops.
            pending = None
            for b in range(BLOC):
                for hc in range(NCH):
                    hs = hc * P
                    if pending is not None:
                        nc.scalar.dma_start(*pending)
                    # one 768 KB DMA: channels side by side in the free dim
                    xt = xin.tile([P, C * W], F32, tag="x")
                    xsrc = x[b].rearrange("c (n p) w -> n p c w", p=P)[hc]
                    nc.sync.dma_start(
                        xt[:].rearrange("p (c w) -> p c w", w=W), xsrc
                    )
                    x0 = xt[:, 0 * W:1 * W]
                    x1 = xt[:, 1 * W:2 * W]
                    x2 = xt[:, 2 * W:3 * W]
                    # grayscale split across DVE / ACT / Pool so no engine saturates
                    g0 = work.tile([P, W], F32, tag="g0")
                    nc.vector.scalar_tensor_tensor(g0[:], x0, ga, x2, mult, add)
                    gs = work.tile([P, W], F32, tag="gs")
                    nc.scalar.mul(gs[:], x1, gb)
                    g1 = work.tile([P, W], F32, tag="g1")
                    nc.gpsimd.tensor_tensor(g1[:], gs[:], g0[:], add)
                    # H-direction DCT
                    p1 = psum_pool.tile([P, W], F32, tag="p1")
                    nc.tensor.matmul(p1[:], wd[:], g1[:], start=True, stop=True)
                    # 32x32 block transpose straight out of PSUM
                    s1t = work.tile([P, W], F32, tag="s1t")
                    nc.vector.transpose(s1t[:], p1[:])
                    # W-direction DCT
                    p2 = psum_pool.tile([P, W], F32, tag="p2")
                    nc.tensor.matmul(p2[:], wd[:], s1t[:], start=True, stop=True)
                    # high-pass mask + PSUM->SBUF move on ACT: columns with
                    # u<4 get a per-partition 0/1 scale (zero iff v<4), the
                    # u>=4 columns are a plain copy.
                    s2 = work.tile([P, W], F32, tag="s2")
                    p2v = p2[:].rearrange("p (g u) -> p g u", u=8)
                    s2v = s2[:].rearrange("p (g u) -> p g u", u=8)
                    nc.scalar.mul(s2v[:, :, 0:4], p2v[:, :, 0:4], mv[:])
                    nc.scalar.copy(s2v[:, :, 4:8], p2v[:, :, 4:8])
                    # block transpose back to natural layout
                    s2t = work.tile([P, W], F32, tag="s2t")
                    nc.vector.transpose(s2t[:], s2[:])
                    # outputs ride the ACT HWDGE queue; inputs own the SP queue
                    pending = (out[b, 0, hs:hs + P, :], s2t[:])
            nc.scalar.dma_start(*pending)
    nc.compile()
    return nc


def _host_constants(dct_matrix, mask):
    D = np.asarray(dct_matrix, dtype=np.float32)
    M = np.asarray(mask, dtype=np.float32)
    dctT = np.kron(np.eye(P // 8, dtype=np.float32), D.T).astype(np.float32)
    # fold the trailing grayscale scale (GRAY_W[2]) into the (shared) DCT
    # weight as sqrt(c): both matmuls apply it, so the chain gains c total.
    wts = (np.sqrt(np.float32(GRAY_W[2])) * dctT).astype(np.float32)[None]
    # per-partition mask column for the u<4 output columns: M[u<4, v] is
    # constant in u there, so it reduces to a v-indexed 0/1 vector.
    pi = np.arange(P)
    mvec = np.ascontiguousarray(M[0, pi % 8], dtype=np.float32).reshape(P, 1)
    return wts, mvec


def kernel(x, dct_matrix, mask):
    global _NC, LAST_RUN
    x = np.ascontiguousarray(np.asarray(x, dtype=np.float32))
    assert x.shape == (B, C, H, W)
    wts, mvec = _host_constants(dct_matrix, mask)

    if _NC is None:
        _NC = _build_bass()

    in_maps = [
        {"x": np.ascontiguousarray(x[i * BLOC:(i + 1) * BLOC]),
         "wts": wts, "mvec": mvec}
        for i in range(N_CORES)
    ]
    trace = bool(int(os.environ.get("DCT_TRACE", "0")))
    LAST_RUN = run_bass_kernel_spmd(
        _NC, in_maps, list(range(N_CORES)), trace=trace,
    )
    out = np.concatenate([LAST_RUN.results[i]["out"] for i in range(N_CORES)], axis=0)
    return out


# revision 39
# speedup vs baseline: 1.0491x; 1.0491x over previous
"""Trainium2 Bass kernel for blocked-DCT high-frequency extractor.

Computes, for x (64, 3, 512, 512) f32:
  gray = 0.299*R + 0.587*G + 0.114*B                     (B,1,H,W)
  per 8x8 block:  Y = mask * (D @ block @ D.T)           (2D DCT + high-pass)
  output (64, 1, 512, 512) f32

Strategy (pure data parallel over batch, 8 batches/core on 8 cores; the
kernel is HBM-bound: 24 MiB in + 8 MiB out per core, ~298 GB/s/core
achievable with all 8 cores running => ~113 us floor).

Per core, per (batch, 128-row chunk) of the image:
  1. One fused 768 KB DMA on the SP HWDGE queue brings all 3 channel
     chunks into a (128h, 3*512w) tile (2 KB contiguous runs).
  2. Grayscale spread over three engines so none saturates:
     g0 = x0*(w0/w2) + x2 on DVE (scalar_tensor_tensor),
     gs = x1*(w1/w2) on ACT, g1 = g0 + gs on GpSimd.
  3. H-direction DCT: one matmul with sqrt(w2) * (I_16 kron D^T).
  4. DVE stream-transpose (independent 32x32 blocks) read straight from
     PSUM. Because 8 | 32, this puts w%32 (which contains the
     intra-block w index b) on partitions.
  5. W-direction DCT: one matmul with the same stationary weight
     (the two sqrt(w2) factors give the grayscale w2 scale in total).
  6. High-pass mask on ACT as two strided PSUM->SBUF copies: columns
     with u<4 are scaled by a per-partition 0/1 vector (zero iff v<4),
     u>=4 columns are a plain copy. This keeps the mask off the DVE,
     whose two structural transposes are the tightest compute budget.
  7. DVE stream-transpose back -> exact (hfreq, wfreq) output layout.
  8. 256 KB contiguous output DMA on the ACT HWDGE queue (separate
     queue from the input stream).

The 32x32 block transpose is an involution whose block-nesting (8 | 32)
makes both DCT matmuls use the same I_16 kron D^T stationary weight and
lands the final result in natural row-major layout with zero TensorE
transposes.
"""

import os

import numpy as np

import concourse.bacc as bacc
import concourse.mybir as mybir
import concourse.tile as tile
from concourse.bass_utils import run_bass_kernel_spmd

N_CORES = 8
B, C, H, W = 64, 3, 512, 512
BLOC = B // N_CORES  # batches per core
P = 128              # SBUF partitions / chunk height
NCH = H // P         # 128-row chunks per image
F32 = mybir.dt.float32
GRAY_W = (0.299, 0.587, 0.114)

_NC = None          # cached compiled Bass module
LAST_RUN = None     # BassKernelResults of the most recent run (for test.py)


def _build_bass():
    nc = bacc.Bacc(
        "TRN2",
        target_bir_lowering=False,
        debug=False,
        num_devices=N_CORES,
    )
    x = nc.declare_dram_parameter("x", [BLOC, C, H, W], F32, isOutput=False)
    wts = nc.declare_dram_parameter("wts", [1, P, P], F32, isOutput=False)
    mvec = nc.declare_dram_parameter("mvec", [P, 1], F32, isOutput=False)
    out = nc.declare_dram_parameter("out", [BLOC, 1, H, W], F32, isOutput=True)

    # gray = GW[2] * (x0*(w0/w2) + x2  +  x1*(w1/w2)); the GW[2] scale is
    # folded into the mask tile on the host side. The two scaled terms are
    # computed on different engines in parallel, then summed on GpSimd.
    ga = GRAY_W[0] / GRAY_W[2]
    gb = GRAY_W[1] / GRAY_W[2]
    mult = mybir.AluOpType.mult
    add = mybir.AluOpType.add

    with tile.TileContext(nc) as tc:
        with (
            tc.tile_pool(name="consts", bufs=1) as consts,
            tc.tile_pool(name="xin", bufs=8) as xin,
            tc.tile_pool(name="work", bufs=6) as work,
            tc.tile_pool(name="psum", bufs=4, space="PSUM") as psum_pool,
        ):
            wd = consts.tile([P, P], F32, tag="wd")
            nc.sync.dma_start(wd[:], wts[0])
            mv = consts.tile([P, 1], F32, tag="mvec")
            nc.sync.dma_start(mv[:], mvec[:])

            # out-DMA for chunk i is emitted at the top of iteration i+1 so
            # ACT's in-order stream never parks on the wait for DVE's final
            # transpose ahead of the next chunk's compute ops.
            pending = None
            for b in range(BLOC):
                for hc in range(NCH):
                    hs = hc * P
                    if pending is not None:
                        nc.scalar.dma_start(*pending)
                    # one 768 KB DMA: channels side by side in the free dim
                    xt = xin.tile([P, C * W], F32, tag="x")
                    xsrc = x[b].rearrange("c (n p) w -> n p c w", p=P)[hc]
                    nc.sync.dma_start(
                        xt[:].rearrange("p (c w) -> p c w", w=W), xsrc
                    )
                    x0 = xt[:, 0 * W:1 * W]
                    x1 = xt[:, 1 * W:2 * W]
                    x2 = xt[:, 2 * W:3 * W]
                    # grayscale split across DVE / ACT / Pool so no engine saturates
                    g0 = work.tile([P, W], F32, tag="g0")
                    nc.vector.scalar_tensor_tensor(g0[:], x0, ga, x2, mult, add)
                    gs = work.tile([P, W], F32, tag="gs")
                    nc.scalar.mul(gs[:], x1, gb)
                    g1 = work.tile([P, W], F32, tag="g1")
                    nc.gpsimd.tensor_tensor(g1[:], gs[:], g0[:], add)
                    # H-direction DCT
                    p1 = psum_pool.tile([P, W], F32, tag="p1")
                    nc.tensor.matmul(p1[:], wd[:], g1[:], start=True, stop=True)
                    # 32x32 block transpose straight out of PSUM
                    s1t = work.tile([P, W], F32, tag="s1t")
                    nc.vector.transpose(s1t[:], p1[:])
                    # W-direction DCT
                    p2 = psum_pool.tile([P, W], F32, tag="p2")
                    nc.tensor.matmul(p2[:], wd[:], s1t[:], start=True, stop=True)
                    # high-pass mask + PSUM->SBUF move on ACT: columns with
                    # u<4 get a per-partition 0/1 scale (zero iff v<4), the
                    # u>=4 columns are a plain copy.
                    s2 = work.tile([P, W], F32, tag="s2")
                    p2v = p2[:].rearrange("p (g u) -> p g u", u=8)
                    s2v = s2[:].rearrange("p (g u) -> p g u", u=8)
                    nc.scalar.mul(s2v[:, :, 0:4], p2v[:, :, 0:4], mv[:])
                    nc.scalar.copy(s2v[:, :, 4:8], p2v[:, :, 4:8])
                    # block transpose back to natural layout
                    s2t = work.tile([P, W], F32, tag="s2t")
                    nc.vector.transpose(s2t[:], s2[:])
                    # outputs ride the ACT HWDGE queue; inputs own the SP queue
                    pending = (out[b, 0, hs:hs + P, :], s2t[:])
            nc.scalar.dma_start(*pending)
    nc.compile()
    return nc


def _host_constants(dct_matrix, mask):
    D = np.asarray(dct_matrix, dtype=np.float32)
    M = np.asarray(mask, dtype=np.float32)
    dctT = np.kron(np.eye(P // 8, dtype=np.float32), D.T).astype(np.float32)
    # fold the trailing grayscale scale (GRAY_W[2]) into the (shared) DCT
    # weight as sqrt(c): both matmuls apply it, so the chain gains c total.
    wts = (np.sqrt(np.float32(GRAY_W[2])) * dctT).astype(np.float32)[None]
    # per-partition mask column for the u<4 output columns: M[u<4, v] is
    # constant in u there, so it reduces to a v-indexed 0/1 vector.
    pi = np.arange(P)
    mvec = np.ascontiguousarray(M[0, pi % 8], dtype=np.float32).reshape(P, 1)
    return wts, mvec


def kernel(x, dct_matrix, mask):
    global _NC, LAST_RUN
    x = np.ascontiguousarray(np.asarray(x, dtype=np.float32))
    assert x.shape == (B, C, H, W)
    wts, mvec = _host_constants(dct_matrix, mask)

    if _NC is None:
        _NC = _build_bass()

    in_maps = [
        {"x": np.ascontiguousarray(x[i * BLOC:(i + 1) * BLOC]),
         "wts": wts, "mvec": mvec}
        for i in range(N_CORES)
    ]
    trace = bool(int(os.environ.get("DCT_TRACE", "0")))
    LAST_RUN = run_bass_kernel_spmd(
        _NC, in_maps, list(range(N_CORES)), trace=trace,
    )
    out = np.concatenate([LAST_RUN.results[i]["out"] for i in range(N_CORES)], axis=0)
    return out
